# revision 14
# baseline (speedup 1.0000x reference)
"""CrossVarianceAttention Trainium2 kernel.

Sharding: data-parallel over batch B=8, one batch element per NeuronCore
(8 cores). Each core computes the full two-branch cross-attention for its
batch element; outputs are gathered (and transposed) on host.

Device layout notes (per core, one batch element):
  - activations [1024, 512] are transposed on-device to [512, 1024]
    ("T layout": feature on partitions) because every matmul contracts
    over features.
  - attention runs per (branch, head) in [k, q] layout (k on partitions)
    so that att @ V needs no transpose; per-q statistics (mean/var of the
    variance-weighting and the softmax denominator) are computed with
    tensor-engine ones/B-matrix reductions, then broadcast across
    partitions with gpsimd partition_broadcast.
  - final out_proj produces out^T [512, 1024]; host transposes back.
"""

import os
import sys
from contextlib import ExitStack

import numpy as np

for _p in ("/opt/trn_rl_repo", "/root/.axon_site/_ro/trn_rl_repo"):
    if os.path.isdir(_p) and _p not in sys.path:
        sys.path.insert(0, _p)

import concourse.bass as bass
import concourse.bacc as bacc
import concourse.mybir as mybir
from concourse import tile
from concourse.bass_utils import run_bass_kernel_spmd
from concourse.masks import make_identity

F32 = mybir.dt.float32
F16 = mybir.dt.float16
BF16 = mybir.dt.bfloat16
AX = mybir.AxisListType
OP = mybir.AluOpType
AF = mybir.ActivationFunctionType

B, N, D = 8, 1024, 512
H, DK = 8, 64
NT = N // 128          # 8 n/k tiles of 128
DB = D // 128          # 4 feature blocks of 128
CH = N // 512          # 2 free-dim chunks of 512 (fp32 matmul N limit)
SCALE = float(np.sqrt(DK))
LN_EPS = 1e-5

W_NAMES = ["q_vis", "k_vis", "v_vis", "q_ir", "k_ir", "v_ir", "out_vis", "out_ir"]


def _emit(ctx: ExitStack, tc: "tile.TileContext", io: dict):
    nc = tc.nc

    const_pool = ctx.enter_context(tc.tile_pool(name="const", bufs=1))
    ident = const_pool.tile([128, 128], F32)
    make_identity(nc, ident[:])
    ones_f16 = const_pool.tile([128, 1], F16)
    nc.vector.memset(ones_f16[:], 1.0)
    ones_bf = const_pool.tile([128, 1], BF16)
    nc.vector.memset(ones_bf[:], 1.0)
    c_eps = const_pool.tile([128, 1], F32)
    nc.vector.memset(c_eps[:], LN_EPS)
    c_half = const_pool.tile([128, 1], F32)
    nc.vector.memset(c_half[:], 0.5)

    # --- load per-feature vectors as [128, DB] columns ---
    def load_cols(name):
        t = const_pool.tile([128, DB], F32, tag=f"col_{name}", name=f"col_{name}")
        nc.sync.dma_start(t[:], io[name][:].rearrange("(a p) -> p a", p=128))
        return t

    cols = {}
    for nm in ["ln1_g", "ln1_b", "ln2_g", "ln2_b"]:
        cols[nm] = load_cols(nm)
    for nm in W_NAMES:
        cols["b_" + nm] = load_cols("b_" + nm)

    # persistent projection outputs
    projT_pool = ctx.enter_context(tc.tile_pool(name="projT", bufs=1))
    QT = {}   # [128, DB, N] f16 : Q^T/SCALE per branch (branch -> tile)
    KT = {}   # [128, DB, N] f16 : K^T per branch
    Vn = {}   # [128, NT, D] bf16: V natural per branch
    for br in ("vis", "ir"):
        QT[br] = projT_pool.tile([128, DB, N], F16, tag=f"QT_{br}", name=f"QT_{br}")
        KT[br] = projT_pool.tile([128, DB, N], F16, tag=f"KT_{br}", name=f"KT_{br}")
        Vn[br] = projT_pool.tile([128, NT, D], BF16, tag=f"V_{br}", name=f"V_{br}")

    ot_pool = ctx.enter_context(tc.tile_pool(name="ot", bufs=1))

    stats_pool = ctx.enter_context(tc.tile_pool(name="stats", bufs=1))
    mu16 = {br: stats_pool.tile([H, N], F16, tag=f"mu16_{br}", name=f"mu16_{br}") for br in ("vis", "ir")}

    # =================== Stage A: LN + transpose inputs ===================
    with ExitStack() as sctx:
        inT_pool = sctx.enter_context(tc.tile_pool(name="inT", bufs=1))
        xT = {}
        for nm in ("lnT_vis", "lnT_ir", "fusT_rgb", "fusT_ir"):
            xT[nm] = inT_pool.tile([128, DB, N], F32, tag=nm, name=nm)

        a_pool = sctx.enter_context(tc.tile_pool(name="stA", bufs=4))
        st_pool = sctx.enter_context(tc.tile_pool(name="stA_stats", bufs=8))
        pT_pool = sctx.enter_context(
            tc.tile_pool(name="stA_psum", bufs=2, space="PSUM")
        )

        def ln_transpose(src_ap, gname, bname, dst):
            g, b = cols[gname], cols[bname]
            for nt in range(NT):
                x = a_pool.tile([128, D], F32, tag="x_in")
                nc.sync.dma_start(x[:], src_ap[nt * 128:(nt + 1) * 128, :])
                ssum = st_pool.tile([128, 1], F32, tag="ssum")
                nc.vector.tensor_reduce(ssum[:], x[:], AX.X, OP.add)
                sq = a_pool.tile([128, D], F32, tag="sq_scratch")
                sqsum = st_pool.tile([128, 1], F32, tag="sqsum")
                nc.scalar.activation(sq[:], x[:], AF.Square, accum_out=sqsum[:])
                mu = st_pool.tile([128, 1], F32, tag="mu")
                nc.vector.tensor_scalar_mul(mu[:], ssum[:], 1.0 / D)
                ex2 = st_pool.tile([128, 1], F32, tag="ex2")
                nc.vector.tensor_scalar_mul(ex2[:], sqsum[:], 1.0 / D)
                mu2 = st_pool.tile([128, 1], F32, tag="mu2")
                nc.vector.tensor_mul(mu2[:], mu[:], mu[:])
                var = st_pool.tile([128, 1], F32, tag="var")
                nc.vector.tensor_sub(var[:], ex2[:], mu2[:])
                std = st_pool.tile([128, 1], F32, tag="std")
                nc.scalar.activation(std[:], var[:], AF.Sqrt, bias=c_eps[:])
                rstd = st_pool.tile([128, 1], F32, tag="rstd")
                nc.vector.reciprocal(rstd[:], std[:])
                xh = a_pool.tile([128, D], F32, tag="xhat")
                nc.vector.tensor_scalar(
                    xh[:], x[:], mu[:], rstd[:], OP.subtract, OP.mult
                )
                for kb in range(DB):
                    ps = pT_pool.tile([128, 128], F32, tag="pT")
                    nc.tensor.transpose(
                        ps[:], xh[:, kb * 128:(kb + 1) * 128], ident[:]
                    )
                    nc.vector.tensor_scalar(
                        dst[:, kb, nt * 128:(nt + 1) * 128],
                        ps[:],
                        g[:, kb:kb + 1],
                        b[:, kb:kb + 1],
                        OP.mult,
                        OP.add,
                    )

        def plain_transpose(src_ap, dst):
            for nt in range(NT):
                x = a_pool.tile([128, D], F32, tag="x_in")
                nc.sync.dma_start(x[:], src_ap[nt * 128:(nt + 1) * 128, :])
                for kb in range(DB):
                    ps = pT_pool.tile([128, 128], F32, tag="pT")
                    nc.tensor.transpose(
                        ps[:], x[:, kb * 128:(kb + 1) * 128], ident[:]
                    )
                    nc.scalar.copy(dst[:, kb, nt * 128:(nt + 1) * 128], ps[:])

        ln_transpose(io["rgb_fea"][:], "ln1_g", "ln1_b", xT["lnT_vis"])
        ln_transpose(io["ir_fea"][:], "ln2_g", "ln2_b", xT["lnT_ir"])
        plain_transpose(io["rgb_fused"][:], xT["fusT_rgb"])
        plain_transpose(io["ir_fused"][:], xT["fusT_ir"])

        # =================== Stage B: projections ===================
        w_pool = sctx.enter_context(tc.tile_pool(name="wts", bufs=2))
        pj_pool = sctx.enter_context(
            tc.tile_pool(name="stB_psum", bufs=4, space="PSUM")
        )

        def load_w(name):
            w = w_pool.tile([128, DB, D], F32, tag="W")
            nc.sync.dma_start(
                w[:], io["W_" + name][:].rearrange("(a p) o -> p a o", p=128)
            )
            return w

        def proj_T(xt, wname, dst, scale=None):
            w = load_w(wname)
            bc = cols["b_" + wname]
            for m in range(DB):
                for c in range(CH):
                    ps = pj_pool.tile([128, 512], F32, tag="pj")
                    for kb in range(DB):
                        nc.tensor.matmul(
                            ps[:],
                            w[:, kb, m * 128:(m + 1) * 128],
                            xt[:, kb, c * 512:(c + 1) * 512],
                            start=(kb == 0),
                            stop=(kb == DB - 1),
                        )
                    if scale is None:
                        nc.vector.tensor_scalar_add(
                            dst[:, m, c * 512:(c + 1) * 512], ps[:], bc[:, m:m + 1]
                        )
                    else:
                        nc.vector.tensor_scalar(
                            dst[:, m, c * 512:(c + 1) * 512],
                            ps[:],
                            bc[:, m:m + 1],
                            float(scale),
                            OP.add,
                            OP.mult,
                        )

        def proj_N(xt, wname, dst):
            # natural-layout projection (for V), bias deferred to host-side
            # identity:   sum_k P[k,q] = 1  =>  bias handled via +b after
            # normalization (added on device in out-proj stage via W^T b).
            w = load_w(wname)
            for nt in range(NT):
                ps = pj_pool.tile([128, 512], F32, tag="pj")
                for kb in range(DB):
                    nc.tensor.matmul(
                        ps[:],
                        xt[:, kb, nt * 128:(nt + 1) * 128],
                        w[:, kb, :],
                        start=(kb == 0),
                        stop=(kb == DB - 1),
                    )
                nc.vector.tensor_copy(dst[:, nt, :], ps[:])

        # branch "vis": Q from ir_fused (W_q_ir), K/V from LN(rgb_fea)
        proj_T(xT["fusT_ir"], "q_ir", QT["vis"], scale=1.0 / SCALE)
        proj_T(xT["lnT_vis"], "k_vis", KT["vis"])
        proj_N(xT["lnT_vis"], "v_vis", Vn["vis"])
        # branch "ir": Q from rgb_fused (W_q_vis), K/V from LN(ir_fea)
        proj_T(xT["fusT_rgb"], "q_vis", QT["ir"], scale=1.0 / SCALE)
        proj_T(xT["lnT_ir"], "k_ir", KT["ir"])
        proj_N(xT["lnT_ir"], "v_ir", Vn["ir"])

        # ---- Stage B2: column means of scores via B-matrix trick ----
        # mu[h, q] = (sum_k e[k, q]) / N = (ksum_head . Q^T_head)[q] / N
        b2_pool = sctx.enter_context(tc.tile_pool(name="stB2", bufs=1))
        mu_ps_pool = sctx.enter_context(
            tc.tile_pool(name="stB2_psum", bufs=1, space="PSUM")
        )
        for br in ("vis", "ir"):
            ks = b2_pool.tile([128, DB], F32, tag="ksum")
            for kb in range(DB):
                nc.vector.tensor_reduce(
                    ks[:, kb:kb + 1], KT[br][:, kb, :], AX.X, OP.add
                )
            bmat = b2_pool.tile([128, DB, H], F16, tag="bmat")
            nc.vector.memset(bmat[:], 0.0)
            for h in range(H):
                kb_h, base = h // 2, (h % 2) * 64
                nc.vector.tensor_copy(
                    bmat[base:base + 64, kb_h, h:h + 1],
                    ks[base:base + 64, kb_h:kb_h + 1],
                )
            mps = mu_ps_pool.tile([H, N], F32, tag="mu_ps")
            for c in range(CH):
                for kb in range(DB):
                    nc.tensor.matmul(
                        mps[:, c * 512:(c + 1) * 512],
                        bmat[:, kb, :],
                        QT[br][:, kb, c * 512:(c + 1) * 512],
                        start=(kb == 0),
                        stop=(kb == DB - 1),
                    )
            nc.vector.tensor_scalar_mul(mu16[br][:], mps[:], 1.0 / N)

    # =================== Stage C: attention per (branch, head) ============
    OT = {br: ot_pool.tile([128, DB, N], F16, tag=f"OT_{br}", name=f"OT_{br}") for br in ("vis", "ir")}
    cctx = ctx.enter_context(ExitStack())
    c_pool = cctx.enter_context(tc.tile_pool(name="stC", bufs=2))
    c2_pool = cctx.enter_context(tc.tile_pool(name="stC_e", bufs=6))
    bmu_pool = cctx.enter_context(tc.tile_pool(name="stC_bmu", bufs=4))
    row_pool = cctx.enter_context(tc.tile_pool(name="stC_rows", bufs=3))
    eps_pool = cctx.enter_context(tc.tile_pool(name="e_psum", bufs=4, space="PSUM"))
    aux_pool = cctx.enter_context(tc.tile_pool(name="aux_psum", bufs=2, space="PSUM"))
    o_pool = cctx.enter_context(tc.tile_pool(name="o_psum", bufs=2, space="PSUM"))

    def head_ctx(br, h):
        kb_h, base = h // 2, (h % 2) * 64
        return (KT[br][base:base + 64, kb_h, :], QT[br][base:base + 64, kb_h, :],
                kb_h, base)

    HN = 512  # q-half width: chains run per (head, q-half) for deep pipelining

    for br in ("vis", "ir"):
        for j in range(H // 2):
            pair = (2 * j, 2 * j + 1)
            bmu = {}
            for h in pair:
                for g in range(2):
                    mu_row = row_pool.tile([1, HN], F16, tag="mu_row")
                    nc.sync.dma_start(
                        mu_row[:], mu16[br][h:h + 1, g * HN:(g + 1) * HN]
                    )
                    bmu[h, g] = bmu_pool.tile([128, HN], F16, tag="bmu",
                                              name="bmu")
                    nc.gpsimd.partition_broadcast(bmu[h, g][:], mu_row[:])

            for g in range(2):
                qs = slice(g * HN, (g + 1) * HN)
                # --- scores, pair-interleaved for PE row-group overlap ---
                e16 = {h: c2_pool.tile([128, NT, HN], F16, tag="e16",
                                       name="e16") for h in pair}
                for kt in range(NT):
                    eps = {}
                    for h in pair:
                        kt_h, qt_h, _, _ = head_ctx(br, h)
                        eps[h] = eps_pool.tile([128, HN], F32, tag="e_ps",
                                               name="e_ps")
                        nc.tensor.matmul(
                            eps[h][:],
                            kt_h[:, kt * 128:(kt + 1) * 128],
                            qt_h[:, qs],
                            start=True,
                            stop=True,
                        )
                    for h in pair:
                        nc.scalar.copy(e16[h][:, kt, :], eps[h][:])

                # --- in-place chain ---
                tcx = {h: c_pool.tile([128, NT, HN], F16, tag="tc", name="tc",
                                      bufs=4) for h in pair}
                for h in pair:
                    bmu_b = bmu[h, g][:].rearrange(
                        "p (o n) -> p o n", o=1).to_broadcast([128, NT, HN])
                    nc.vector.tensor_sub(tcx[h][:], e16[h][:], bmu_b)
                for h in pair:
                    nc.vector.tensor_mul(tcx[h][:], tcx[h][:], tcx[h][:])
                brx = {}
                for h in pair:
                    vps = aux_pool.tile([1, HN], F32, tag="red_ps",
                                        name="red_ps")
                    for kt in range(NT):
                        nc.tensor.matmul(
                            vps[:],
                            ones_f16[:],
                            tcx[h][:, kt, :],
                            start=(kt == 0),
                            stop=(kt == NT - 1),
                        )
                    rr = row_pool.tile([1, HN], F32, tag="rr")
                    nc.vector.tensor_scalar(
                        rr[:], vps[:], 2.0 / N, 1e-6, OP.mult, OP.add
                    )
                    rf = row_pool.tile([1, HN], F32, tag="rf")
                    nc.vector.reciprocal_approx_fast(rf[:], rr[:])
                    r16row = row_pool.tile([1, HN], F16, tag="r16row")
                    nc.vector.tensor_copy(r16row[:], rf[:])
                    brx[h] = c_pool.tile([128, HN], F16, tag="br16",
                                         name="br16", bufs=4)
                    nc.gpsimd.partition_broadcast(brx[h][:], r16row[:])
                for h in pair:
                    br_b = brx[h][:].rearrange(
                        "p (o n) -> p o n", o=1).to_broadcast([128, NT, HN])
                    nc.vector.tensor_mul(tcx[h][:], tcx[h][:], br_b)
                for h in pair:
                    nc.scalar.activation(tcx[h][:], tcx[h][:], AF.Sigmoid,
                                         bias=c_half[:])
                for h in pair:
                    nc.vector.tensor_mul(tcx[h][:], e16[h][:], tcx[h][:])
                ew = {}
                for h in pair:
                    ew[h] = c2_pool.tile([128, NT, HN], BF16, tag="e16",
                                         name="ew")
                    nc.scalar.activation(ew[h][:], tcx[h][:], AF.Exp)

                # --- softmax denominator ---
                brd = {}
                for h in pair:
                    dps = aux_pool.tile([1, HN], F32, tag="red_ps",
                                        name="red_ps")
                    for kt in range(NT):
                        nc.tensor.matmul(
                            dps[:],
                            ones_bf[:],
                            ew[h][:, kt, :],
                            start=(kt == 0),
                            stop=(kt == NT - 1),
                        )
                    dd = row_pool.tile([1, HN], F32, tag="rr")
                    nc.vector.tensor_copy(dd[:], dps[:])
                    rd = row_pool.tile([1, HN], F32, tag="rf")
                    nc.vector.reciprocal_approx_fast(rd[:], dd[:])
                    brd[h] = c_pool.tile([128, HN], F32, tag="brd",
                                         name="brd", bufs=3)
                    nc.gpsimd.partition_broadcast(brd[h][:], rd[:])

                # --- AV, pair-packed into psum column groups ---
                ops = o_pool.tile([128, HN], F32, tag="o_ps", name="o_ps")
                for kt in range(NT):
                    for h in pair:
                        base_o = (h % 2) * 64
                        nc.tensor.matmul(
                            ops[base_o:base_o + 64, :],
                            Vn[br][:, kt, h * 64:(h + 1) * 64],
                            ew[h][:, kt, :],
                            start=(kt == 0),
                            stop=(kt == NT - 1),
                            tile_position=(0, base_o),
                            skip_group_check=True,
                        )
                for h in pair:
                    _, _, kb_h, base = head_ctx(br, h)
                    base_o = (h % 2) * 64
                    nc.vector.scalar_tensor_tensor(
                        OT[br][base:base + 64, kb_h, qs],
                        ops[base_o:base_o + 64, :],
                        1.0,
                        brd[h][:64, :],
                        OP.mult,
                        OP.mult,
                    )

    cctx.close()

    # =================== Stage D: out-proj (transposed output) ============
    with ExitStack() as sctx:
        w_pool = sctx.enter_context(tc.tile_pool(name="wts_out", bufs=2))
        d_pool = sctx.enter_context(tc.tile_pool(name="stD", bufs=4))
        dp_pool = sctx.enter_context(
            tc.tile_pool(name="stD_psum", bufs=4, space="PSUM")
        )
        for br in ("vis", "ir"):
            wname = "out_" + br
            w32 = w_pool.tile([128, DB, D], F32, tag="Wout32")
            nc.sync.dma_start(
                w32[:], io["W_" + wname][:].rearrange("(a p) o -> p a o", p=128)
            )
            w = w_pool.tile([128, DB, D], F16, tag="Wout")
            nc.vector.tensor_copy(w[:], w32[:])
            bout = cols["b_" + wname]
            bv = cols["b_v_" + br]
            # total bias = b_out + W_out^T b_v   (V-projection bias folded in)
            btot = d_pool.tile([128, DB], F32, tag="btot")
            for m in range(DB):
                wb = dp_pool.tile([128, 1], F32, tag="wb_ps")
                for kb in range(DB):
                    nc.tensor.matmul(
                        wb[:],
                        w32[:, kb, m * 128:(m + 1) * 128],
                        bv[:, kb:kb + 1],
                        start=(kb == 0),
                        stop=(kb == DB - 1),
                    )
                nc.vector.tensor_add(btot[:, m:m + 1], wb[:], bout[:, m:m + 1])
            out_dram = io["out_vis_T"] if br == "vis" else io["out_ir_T"]
            for m in range(DB):
                for c in range(CH):
                    ps = dp_pool.tile([128, 512], F32, tag="op_ps")
                    for kb in range(DB):
                        nc.tensor.matmul(
                            ps[:],
                            w[:, kb, m * 128:(m + 1) * 128],
                            OT[br][:, kb, c * 512:(c + 1) * 512],
                            start=(kb == 0),
                            stop=(kb == DB - 1),
                        )
                    osb = d_pool.tile([128, 512], F32, tag="osb")
                    nc.vector.tensor_scalar_add(osb[:], ps[:], btot[:, m:m + 1])
                    nc.sync.dma_start(
                        out_dram[m * 128:(m + 1) * 128, c * 512:(c + 1) * 512],
                        osb[:],
                    )


def build_nc():
    nc = bacc.Bacc()
    io = {}
    for nm in ["rgb_fea", "ir_fea", "rgb_fused", "ir_fused"]:
        io[nm] = nc.declare_dram_parameter(nm, [N, D], F32, isOutput=False)
    for nm in W_NAMES:
        io["W_" + nm] = nc.declare_dram_parameter("W_" + nm, [D, D], F32, isOutput=False)
        io["b_" + nm] = nc.declare_dram_parameter("b_" + nm, [D], F32, isOutput=False)
    for nm in ["ln1_g", "ln1_b", "ln2_g", "ln2_b"]:
        io[nm] = nc.declare_dram_parameter(nm, [D], F32, isOutput=False)
    io["out_vis_T"] = nc.declare_dram_parameter("out_vis_T", [D, N], F32, isOutput=True)
    io["out_ir_T"] = nc.declare_dram_parameter("out_ir_T", [D, N], F32, isOutput=True)

    with tile.TileContext(nc) as tc:
        with ExitStack() as ctx:
            _emit(ctx, tc, io)
    nc.finalize()
    return nc


_NC_CACHE = None


def _get_nc():
    global _NC_CACHE
    if _NC_CACHE is None:
        _NC_CACHE = build_nc()
    return _NC_CACHE


def _in_maps(rgb_fea, ir_fea, rgb_fused, ir_fused, params):
    maps = []
    for i in range(B):
        m = {
            "rgb_fea": np.ascontiguousarray(rgb_fea[i], np.float32),
            "ir_fea": np.ascontiguousarray(ir_fea[i], np.float32),
            "rgb_fused": np.ascontiguousarray(rgb_fused[i], np.float32),
            "ir_fused": np.ascontiguousarray(ir_fused[i], np.float32),
        }
        for nm in W_NAMES:
            m["W_" + nm] = np.ascontiguousarray(params["W_" + nm], np.float32)
            m["b_" + nm] = np.ascontiguousarray(params["b_" + nm], np.float32)
        for nm in ["ln1_g", "ln1_b", "ln2_g", "ln2_b"]:
            m[nm] = np.ascontiguousarray(params[nm], np.float32)
        maps.append(m)
    return maps


def run(rgb_fea, ir_fea, rgb_fused, ir_fused, params, trace=False):
    nc = _get_nc()
    maps = _in_maps(
        np.asarray(rgb_fea), np.asarray(ir_fea),
        np.asarray(rgb_fused), np.asarray(ir_fused), params,
    )
    res = run_bass_kernel_spmd(nc, maps, list(range(B)), trace=trace)
    out_vis = np.stack([res.results[i]["out_vis_T"].T for i in range(B)])
    out_ir = np.stack([res.results[i]["out_ir_T"].T for i in range(B)])
    return (out_vis, out_ir), res


def kernel(rgb_fea, ir_fea, rgb_fused, ir_fused, params):
    (out_vis, out_ir), _ = run(rgb_fea, ir_fea, rgb_fused, ir_fused, params)
    return out_vis, out_ir


# revision 18
# speedup vs baseline: 1.0310x; 1.0310x over previous
"""CrossVarianceAttention Trainium2 kernel.

Sharding: data-parallel over batch B=8, one batch element per NeuronCore
(8 cores). Each core computes the full two-branch cross-attention for its
batch element; outputs are gathered (and transposed) on host.

Device layout notes (per core, one batch element):
  - activations [1024, 512] are transposed on-device to [512, 1024]
    ("T layout": feature on partitions) because every matmul contracts
    over features.
  - attention runs per (branch, head) in [k, q] layout (k on partitions)
    so that att @ V needs no transpose; per-q statistics (mean/var of the
    variance-weighting and the softmax denominator) are computed with
    tensor-engine ones/B-matrix reductions, then broadcast across
    partitions with gpsimd partition_broadcast.
  - final out_proj produces out^T [512, 1024]; host transposes back.
"""

import os
import sys
from contextlib import ExitStack

import numpy as np

for _p in ("/opt/trn_rl_repo", "/root/.axon_site/_ro/trn_rl_repo"):
    if os.path.isdir(_p) and _p not in sys.path:
        sys.path.insert(0, _p)

import concourse.bass as bass
import concourse.bacc as bacc
import concourse.mybir as mybir
from concourse import tile
from concourse.bass_utils import run_bass_kernel_spmd
from concourse.masks import make_identity

F32 = mybir.dt.float32
F16 = mybir.dt.float16
BF16 = mybir.dt.bfloat16
AX = mybir.AxisListType
OP = mybir.AluOpType
AF = mybir.ActivationFunctionType

B, N, D = 8, 1024, 512
H, DK = 8, 64
NT = N // 128          # 8 n/k tiles of 128
DB = D // 128          # 4 feature blocks of 128
CH = N // 512          # 2 free-dim chunks of 512 (fp32 matmul N limit)
SCALE = float(np.sqrt(DK))
LN_EPS = 1e-5

W_NAMES = ["q_vis", "k_vis", "v_vis", "q_ir", "k_ir", "v_ir", "out_vis", "out_ir"]


def _emit(ctx: ExitStack, tc: "tile.TileContext", io: dict):
    nc = tc.nc

    const_pool = ctx.enter_context(tc.tile_pool(name="const", bufs=1))
    ident = const_pool.tile([128, 128], F32)
    make_identity(nc, ident[:])
    ones_f16 = const_pool.tile([128, 1], F16)
    nc.vector.memset(ones_f16[:], 1.0)
    ones_bf = const_pool.tile([128, 1], BF16)
    nc.vector.memset(ones_bf[:], 1.0)
    c_eps = const_pool.tile([128, 1], F32)
    nc.vector.memset(c_eps[:], LN_EPS)
    c_half = const_pool.tile([128, 1], F32)
    nc.vector.memset(c_half[:], 0.5)

    # --- load per-feature vectors as [128, DB] columns ---
    def load_cols(name):
        t = const_pool.tile([128, DB], F32, tag=f"col_{name}", name=f"col_{name}")
        nc.sync.dma_start(t[:], io[name][:].rearrange("(a p) -> p a", p=128))
        return t

    cols = {}
    for nm in ["ln1_g", "ln1_b", "ln2_g", "ln2_b"]:
        cols[nm] = load_cols(nm)
    for nm in W_NAMES:
        cols["b_" + nm] = load_cols("b_" + nm)

    # persistent projection outputs
    projT_pool = ctx.enter_context(tc.tile_pool(name="projT", bufs=1))
    QT = {}   # [128, DB, N] f16 : Q^T/SCALE per branch (branch -> tile)
    KT = {}   # [128, DB, N] f16 : K^T per branch
    Vn = {}   # [128, NT, D] bf16: V natural per branch
    for br in ("vis", "ir"):
        QT[br] = projT_pool.tile([128, DB, N], F16, tag=f"QT_{br}", name=f"QT_{br}")
        KT[br] = projT_pool.tile([128, DB, N], F16, tag=f"KT_{br}", name=f"KT_{br}")
        Vn[br] = projT_pool.tile([128, NT, D], BF16, tag=f"V_{br}", name=f"V_{br}")

    ot_pool = ctx.enter_context(tc.tile_pool(name="ot", bufs=1))

    stats_pool = ctx.enter_context(tc.tile_pool(name="stats", bufs=1))
    mu16 = {br: stats_pool.tile([H, N], F16, tag=f"mu16_{br}", name=f"mu16_{br}") for br in ("vis", "ir")}

    # ============ Stage A: LN (gamma/beta folded into W) + DMA-xbar T ======
    with ExitStack() as sctx:
        inT_pool = sctx.enter_context(tc.tile_pool(name="inT", bufs=1))
        xT = {}
        for nm in ("lnT_vis", "lnT_ir", "fusT_rgb", "fusT_ir"):
            xT[nm] = inT_pool.tile([128, DB, N], F16, tag=nm, name=nm)

        a_pool = sctx.enter_context(tc.tile_pool(name="stA", bufs=4))
        st_pool = sctx.enter_context(tc.tile_pool(name="stA_stats", bufs=8))

        def ln_transpose(src_ap, dst):
            # xhat = (x - mean) * rstd  (gamma/beta folded into K/V weights)
            for nt in range(NT):
                x = a_pool.tile([128, D], F32, tag="x_in")
                nc.sync.dma_start(x[:], src_ap[nt * 128:(nt + 1) * 128, :])
                ssum = st_pool.tile([128, 1], F32, tag="ssum")
                nc.vector.tensor_reduce(ssum[:], x[:], AX.X, OP.add)
                sq = a_pool.tile([128, D], F32, tag="sq_scratch")
                sqsum = st_pool.tile([128, 1], F32, tag="sqsum")
                nc.scalar.activation(sq[:], x[:], AF.Square, accum_out=sqsum[:])
                mu = st_pool.tile([128, 1], F32, tag="mu")
                nc.vector.tensor_scalar_mul(mu[:], ssum[:], 1.0 / D)
                ex2 = st_pool.tile([128, 1], F32, tag="ex2")
                nc.vector.tensor_scalar_mul(ex2[:], sqsum[:], 1.0 / D)
                mu2 = st_pool.tile([128, 1], F32, tag="mu2")
                nc.vector.tensor_mul(mu2[:], mu[:], mu[:])
                var = st_pool.tile([128, 1], F32, tag="var")
                nc.vector.tensor_sub(var[:], ex2[:], mu2[:])
                std = st_pool.tile([128, 1], F32, tag="std")
                nc.scalar.activation(std[:], var[:], AF.Sqrt, bias=c_eps[:])
                rstd = st_pool.tile([128, 1], F32, tag="rstd")
                nc.vector.reciprocal(rstd[:], std[:])
                xh = a_pool.tile([128, D], F16, tag="xhat")
                nc.vector.tensor_scalar(
                    xh[:], x[:], mu[:], rstd[:], OP.subtract, OP.mult
                )
                nc.sync.dma_start_transpose(
                    dst[:, :, nt * 128:(nt + 1) * 128], xh[:]
                )

        def plain_transpose(src_ap, dst):
            for nt in range(NT):
                x = a_pool.tile([128, D], F32, tag="x_in")
                nc.sync.dma_start(x[:], src_ap[nt * 128:(nt + 1) * 128, :])
                x16 = a_pool.tile([128, D], F16, tag="xhat")
                nc.vector.tensor_copy(x16[:], x[:])
                nc.sync.dma_start_transpose(
                    dst[:, :, nt * 128:(nt + 1) * 128], x16[:]
                )

        ln_transpose(io["rgb_fea"][:], xT["lnT_vis"])
        ln_transpose(io["ir_fea"][:], xT["lnT_ir"])
        plain_transpose(io["rgb_fused"][:], xT["fusT_rgb"])
        plain_transpose(io["ir_fused"][:], xT["fusT_ir"])

        # =================== Stage B: projections (fp16) ===================
        w_pool = sctx.enter_context(tc.tile_pool(name="wts", bufs=2))
        w16_pool = sctx.enter_context(tc.tile_pool(name="wts16", bufs=2))
        pj_pool = sctx.enter_context(
            tc.tile_pool(name="stB_psum", bufs=4, space="PSUM")
        )

        def load_w16(name, gcol=None):
            # fp16 weight; optionally scaled by LN gamma (per-din rows)
            w = w_pool.tile([128, DB, D], F32, tag="W")
            nc.sync.dma_start(
                w[:], io["W_" + name][:].rearrange("(a p) o -> p a o", p=128)
            )
            w16 = w16_pool.tile([128, DB, D], F16, tag="W16")
            for kb in range(DB):
                if gcol is None:
                    nc.vector.tensor_copy(w16[:, kb, :], w[:, kb, :])
                else:
                    nc.vector.tensor_scalar_mul(
                        w16[:, kb, :], w[:, kb, :], gcol[:, kb:kb + 1]
                    )
            return w, w16

        def fold_beta_bias(wname, w32, bcol_beta):
            # returns [128, DB] col tile of (b_w + beta^T W); column m of the
            # result is computed as W[:, m-tile]^T beta via matmul (keeps the
            # dout index on partitions, so no cross-partition moves needed)
            btot = const_pool.tile([128, DB], F32, tag=f"btot_{wname}",
                                   name=f"btot_{wname}")
            for m in range(DB):
                bw_ps = pj_pool.tile([128, 1], F32, tag="bw_ps", name="bw_ps",
                                     bufs=2)
                for kb in range(DB):
                    nc.tensor.matmul(
                        bw_ps[:],
                        w32[:, kb, m * 128:(m + 1) * 128],
                        bcol_beta[:, kb:kb + 1],
                        start=(kb == 0),
                        stop=(kb == DB - 1),
                    )
                nc.vector.tensor_tensor(
                    btot[:, m:m + 1], bw_ps[:], cols["b_" + wname][:, m:m + 1],
                    OP.add,
                )
            return btot

        def proj_T(xt, w16, bcol, dst, scale=None):
            for m in range(DB):
                for c in range(CH):
                    ps = pj_pool.tile([128, 512], F32, tag="pj")
                    for kb in range(DB):
                        nc.tensor.matmul(
                            ps[:],
                            w16[:, kb, m * 128:(m + 1) * 128],
                            xt[:, kb, c * 512:(c + 1) * 512],
                            start=(kb == 0),
                            stop=(kb == DB - 1),
                        )
                    if scale is None:
                        nc.vector.tensor_scalar_add(
                            dst[:, m, c * 512:(c + 1) * 512], ps[:],
                            bcol[:, m:m + 1]
                        )
                    else:
                        nc.vector.tensor_scalar(
                            dst[:, m, c * 512:(c + 1) * 512],
                            ps[:],
                            bcol[:, m:m + 1],
                            float(scale),
                            OP.add,
                            OP.mult,
                        )

        def proj_N(xt, w16, dst):
            # natural-layout projection (for V); bias folded into out-proj
            for nt in range(NT):
                ps = pj_pool.tile([128, 512], F32, tag="pj")
                for kb in range(DB):
                    nc.tensor.matmul(
                        ps[:],
                        xt[:, kb, nt * 128:(nt + 1) * 128],
                        w16[:, kb, :],
                        start=(kb == 0),
                        stop=(kb == DB - 1),
                    )
                nc.vector.tensor_copy(dst[:, nt, :], ps[:])

        # branch "vis": Q from ir_fused (W_q_ir), K/V from LN(rgb_fea)
        # branch "ir":  Q from rgb_fused (W_q_vis), K/V from LN(ir_fea)
        for br, qw, qx, kvg, kvb, kw, vw, kvx in (
            ("vis", "q_ir", "fusT_ir", "ln1_g", "ln1_b", "k_vis", "v_vis",
             "lnT_vis"),
            ("ir", "q_vis", "fusT_rgb", "ln2_g", "ln2_b", "k_ir", "v_ir",
             "lnT_ir"),
        ):
            _, wq16 = load_w16(qw)
            proj_T(xT[qx], wq16, cols["b_" + qw], QT[br], scale=1.0 / SCALE)
            wk32, wk16 = load_w16(kw, gcol=cols[kvg])
            btot_k = fold_beta_bias(kw, wk32, cols[kvb])
            proj_T(xT[kvx], wk16, btot_k, KT[br])
            wv32, wv16 = load_w16(vw, gcol=cols[kvg])
            # V bias: (b_v + beta^T W_v) folded into out-proj via W_out^T b
            btot_v = fold_beta_bias(vw, wv32, cols[kvb])
            proj_N(xT[kvx], wv16, Vn[br])
            cols["btot_v_" + br] = btot_v

        # ---- Stage B2: column means of scores via B-matrix trick ----
        # mu[h, q] = (sum_k e[k, q]) / N = (ksum_head . Q^T_head)[q] / N
        b2_pool = sctx.enter_context(tc.tile_pool(name="stB2", bufs=1))
        mu_ps_pool = sctx.enter_context(
            tc.tile_pool(name="stB2_psum", bufs=1, space="PSUM")
        )
        for br in ("vis", "ir"):
            ks = b2_pool.tile([128, DB], F32, tag="ksum")
            for kb in range(DB):
                nc.vector.tensor_reduce(
                    ks[:, kb:kb + 1], KT[br][:, kb, :], AX.X, OP.add
                )
            bmat = b2_pool.tile([128, DB, H], F16, tag="bmat")
            nc.vector.memset(bmat[:], 0.0)
            for h in range(H):
                kb_h, base = h // 2, (h % 2) * 64
                nc.vector.tensor_copy(
                    bmat[base:base + 64, kb_h, h:h + 1],
                    ks[base:base + 64, kb_h:kb_h + 1],
                )
            mps = mu_ps_pool.tile([H, N], F32, tag="mu_ps")
            for c in range(CH):
                for kb in range(DB):
                    nc.tensor.matmul(
                        mps[:, c * 512:(c + 1) * 512],
                        bmat[:, kb, :],
                        QT[br][:, kb, c * 512:(c + 1) * 512],
                        start=(kb == 0),
                        stop=(kb == DB - 1),
                    )
            nc.vector.tensor_scalar_mul(mu16[br][:], mps[:], 1.0 / N)

    # =================== Stage C: attention per (branch, head) ============
    OT = {br: ot_pool.tile([128, DB, N], F16, tag=f"OT_{br}", name=f"OT_{br}") for br in ("vis", "ir")}
    cctx = ctx.enter_context(ExitStack())
    c_pool = cctx.enter_context(tc.tile_pool(name="stC", bufs=2))
    c2_pool = cctx.enter_context(tc.tile_pool(name="stC_e", bufs=6))
    bmu_pool = cctx.enter_context(tc.tile_pool(name="stC_bmu", bufs=4))
    row_pool = cctx.enter_context(tc.tile_pool(name="stC_rows", bufs=3))
    eps_pool = cctx.enter_context(tc.tile_pool(name="e_psum", bufs=3, space="PSUM"))
    aux_pool = cctx.enter_context(tc.tile_pool(name="aux_psum", bufs=4, space="PSUM"))
    o_pool = cctx.enter_context(tc.tile_pool(name="o_psum", bufs=1, space="PSUM"))

    def head_ctx(br, h):
        kb_h, base = h // 2, (h % 2) * 64
        return (KT[br][base:base + 64, kb_h, :], QT[br][base:base + 64, kb_h, :],
                kb_h, base)

    HN = 512  # q-half width: chains run per (head, q-half) for deep pipelining

    for br in ("vis", "ir"):
        for j in range(H // 2):
            pair = (2 * j, 2 * j + 1)
            bmu = {}
            for h in pair:
                for g in range(2):
                    mu_row = row_pool.tile([1, HN], F16, tag="mu_row")
                    nc.sync.dma_start(
                        mu_row[:], mu16[br][h:h + 1, g * HN:(g + 1) * HN]
                    )
                    bmu[h, g] = bmu_pool.tile([128, HN], F16, tag="bmu",
                                              name="bmu")
                    nc.gpsimd.partition_broadcast(bmu[h, g][:], mu_row[:])

            for g in range(2):
                qs = slice(g * HN, (g + 1) * HN)
                # --- scores, pair-interleaved for PE row-group overlap ---
                e16 = {h: c2_pool.tile([128, NT, HN], F16, tag="e16",
                                       name="e16") for h in pair}
                for kt in range(NT):
                    eps = {}
                    for h in pair:
                        kt_h, qt_h, _, _ = head_ctx(br, h)
                        eps[h] = eps_pool.tile([128, HN], F32, tag="e_ps",
                                               name="e_ps")
                        nc.tensor.matmul(
                            eps[h][:],
                            kt_h[:, kt * 128:(kt + 1) * 128],
                            qt_h[:, qs],
                            start=True,
                            stop=True,
                        )
                    for h in pair:
                        nc.scalar.copy(e16[h][:, kt, :], eps[h][:])

                # --- in-place chain ---
                tcx = {h: c_pool.tile([128, NT, HN], F16, tag="tc", name="tc",
                                      bufs=4) for h in pair}
                for h in pair:
                    bmu_b = bmu[h, g][:].rearrange(
                        "p (o n) -> p o n", o=1).to_broadcast([128, NT, HN])
                    nc.vector.tensor_sub(tcx[h][:], e16[h][:], bmu_b)
                for h in pair:
                    nc.vector.tensor_mul(tcx[h][:], tcx[h][:], tcx[h][:])
                brx = {}
                for h in pair:
                    vps = aux_pool.tile([1, HN], F32, tag="red_ps",
                                        name="red_ps")
                    for kt in range(NT):
                        nc.tensor.matmul(
                            vps[:],
                            ones_f16[:],
                            tcx[h][:, kt, :],
                            start=(kt == 0),
                            stop=(kt == NT - 1),
                        )
                    rr = row_pool.tile([1, HN], F32, tag="rr")
                    nc.vector.tensor_scalar(
                        rr[:], vps[:], 2.0 / N, 1e-6, OP.mult, OP.add
                    )
                    rf = row_pool.tile([1, HN], F32, tag="rf")
                    nc.vector.reciprocal_approx_fast(rf[:], rr[:])
                    r16row = row_pool.tile([1, HN], F16, tag="r16row")
                    nc.vector.tensor_copy(r16row[:], rf[:])
                    brx[h] = c_pool.tile([128, HN], F16, tag="br16",
                                         name="br16", bufs=4)
                    nc.gpsimd.partition_broadcast(brx[h][:], r16row[:])
                for h in pair:
                    br_b = brx[h][:].rearrange(
                        "p (o n) -> p o n", o=1).to_broadcast([128, NT, HN])
                    nc.vector.tensor_mul(tcx[h][:], tcx[h][:], br_b)
                for h in pair:
                    nc.scalar.activation(tcx[h][:], tcx[h][:], AF.Sigmoid,
                                         bias=c_half[:])
                for h in pair:
                    nc.vector.tensor_mul(tcx[h][:], e16[h][:], tcx[h][:])
                ew = {}
                for h in pair:
                    ew[h] = c2_pool.tile([128, NT, HN], BF16, tag="e16",
                                         name="ew")
                    nc.scalar.activation(ew[h][:], tcx[h][:], AF.Exp)

                # --- softmax denominator ---
                brd = {}
                for h in pair:
                    dps = aux_pool.tile([1, HN], F32, tag="red_ps",
                                        name="red_ps")
                    for kt in range(NT):
                        nc.tensor.matmul(
                            dps[:],
                            ones_bf[:],
                            ew[h][:, kt, :],
                            start=(kt == 0),
                            stop=(kt == NT - 1),
                        )
                    dd = row_pool.tile([1, HN], F32, tag="rr")
                    nc.vector.tensor_copy(dd[:], dps[:])
                    rd = row_pool.tile([1, HN], F32, tag="rf")
                    nc.vector.reciprocal_approx_fast(rd[:], dd[:])
                    brd[h] = c_pool.tile([128, HN], F32, tag="brd",
                                         name="brd", bufs=3)
                    nc.gpsimd.partition_broadcast(brd[h][:], rd[:])

                # --- AV, pair-packed into psum column groups ---
                ops = o_pool.tile([128, HN], F32, tag="o_ps", name="o_ps")
                for kt in range(NT):
                    for h in pair:
                        base_o = (h % 2) * 64
                        nc.tensor.matmul(
                            ops[base_o:base_o + 64, :],
                            Vn[br][:, kt, h * 64:(h + 1) * 64],
                            ew[h][:, kt, :],
                            start=(kt == 0),
                            stop=(kt == NT - 1),
                            tile_position=(0, base_o),
                            skip_group_check=True,
                        )
                for h in pair:
                    _, _, kb_h, base = head_ctx(br, h)
                    base_o = (h % 2) * 64
                    nc.vector.scalar_tensor_tensor(
                        OT[br][base:base + 64, kb_h, qs],
                        ops[base_o:base_o + 64, :],
                        1.0,
                        brd[h][:64, :],
                        OP.mult,
                        OP.mult,
                    )

    cctx.close()

    # =================== Stage D: out-proj (transposed output) ============
    with ExitStack() as sctx:
        w_pool = sctx.enter_context(tc.tile_pool(name="wts_out", bufs=2))
        d_pool = sctx.enter_context(tc.tile_pool(name="stD", bufs=4))
        dp_pool = sctx.enter_context(
            tc.tile_pool(name="stD_psum", bufs=4, space="PSUM")
        )
        for br in ("vis", "ir"):
            wname = "out_" + br
            w32 = w_pool.tile([128, DB, D], F32, tag="Wout32")
            nc.sync.dma_start(
                w32[:], io["W_" + wname][:].rearrange("(a p) o -> p a o", p=128)
            )
            w = w_pool.tile([128, DB, D], F16, tag="Wout")
            nc.vector.tensor_copy(w[:], w32[:])
            bout = cols["b_" + wname]
            bv = cols["btot_v_" + br]
            # total bias = b_out + W_out^T b_v   (V-projection bias folded in)
            btot = d_pool.tile([128, DB], F32, tag="btot")
            for m in range(DB):
                wb = dp_pool.tile([128, 1], F32, tag="wb_ps")
                for kb in range(DB):
                    nc.tensor.matmul(
                        wb[:],
                        w32[:, kb, m * 128:(m + 1) * 128],
                        bv[:, kb:kb + 1],
                        start=(kb == 0),
                        stop=(kb == DB - 1),
                    )
                nc.vector.tensor_add(btot[:, m:m + 1], wb[:], bout[:, m:m + 1])
            out_dram = io["out_vis_T"] if br == "vis" else io["out_ir_T"]
            for m in range(DB):
                for c in range(CH):
                    ps = dp_pool.tile([128, 512], F32, tag="op_ps")
                    for kb in range(DB):
                        nc.tensor.matmul(
                            ps[:],
                            w[:, kb, m * 128:(m + 1) * 128],
                            OT[br][:, kb, c * 512:(c + 1) * 512],
                            start=(kb == 0),
                            stop=(kb == DB - 1),
                        )
                    osb = d_pool.tile([128, 512], F32, tag="osb")
                    nc.vector.tensor_scalar_add(osb[:], ps[:], btot[:, m:m + 1])
                    nc.sync.dma_start(
                        out_dram[m * 128:(m + 1) * 128, c * 512:(c + 1) * 512],
                        osb[:],
                    )


def build_nc():
    nc = bacc.Bacc()
    io = {}
    for nm in ["rgb_fea", "ir_fea", "rgb_fused", "ir_fused"]:
        io[nm] = nc.declare_dram_parameter(nm, [N, D], F32, isOutput=False)
    for nm in W_NAMES:
        io["W_" + nm] = nc.declare_dram_parameter("W_" + nm, [D, D], F32, isOutput=False)
        io["b_" + nm] = nc.declare_dram_parameter("b_" + nm, [D], F32, isOutput=False)
    for nm in ["ln1_g", "ln1_b", "ln2_g", "ln2_b"]:
        io[nm] = nc.declare_dram_parameter(nm, [D], F32, isOutput=False)
    io["out_vis_T"] = nc.declare_dram_parameter("out_vis_T", [D, N], F32, isOutput=True)
    io["out_ir_T"] = nc.declare_dram_parameter("out_ir_T", [D, N], F32, isOutput=True)

    with tile.TileContext(nc) as tc:
        with ExitStack() as ctx:
            _emit(ctx, tc, io)
    nc.finalize()
    return nc


_NC_CACHE = None


def _get_nc():
    global _NC_CACHE
    if _NC_CACHE is None:
        _NC_CACHE = build_nc()
    return _NC_CACHE


def _in_maps(rgb_fea, ir_fea, rgb_fused, ir_fused, params):
    maps = []
    for i in range(B):
        m = {
            "rgb_fea": np.ascontiguousarray(rgb_fea[i], np.float32),
            "ir_fea": np.ascontiguousarray(ir_fea[i], np.float32),
            "rgb_fused": np.ascontiguousarray(rgb_fused[i], np.float32),
            "ir_fused": np.ascontiguousarray(ir_fused[i], np.float32),
        }
        for nm in W_NAMES:
            m["W_" + nm] = np.ascontiguousarray(params["W_" + nm], np.float32)
            m["b_" + nm] = np.ascontiguousarray(params["b_" + nm], np.float32)
        for nm in ["ln1_g", "ln1_b", "ln2_g", "ln2_b"]:
            m[nm] = np.ascontiguousarray(params[nm], np.float32)
        maps.append(m)
    return maps


def run(rgb_fea, ir_fea, rgb_fused, ir_fused, params, trace=False):
    nc = _get_nc()
    maps = _in_maps(
        np.asarray(rgb_fea), np.asarray(ir_fea),
        np.asarray(rgb_fused), np.asarray(ir_fused), params,
    )
    res = run_bass_kernel_spmd(nc, maps, list(range(B)), trace=trace)
    out_vis = np.stack([res.results[i]["out_vis_T"].T for i in range(B)])
    out_ir = np.stack([res.results[i]["out_ir_T"].T for i in range(B)])
    return (out_vis, out_ir), res


def kernel(rgb_fea, ir_fea, rgb_fused, ir_fused, params):
    (out_vis, out_ir), _ = run(rgb_fea, ir_fea, rgb_fused, ir_fused, params)
    return out_vis, out_ir


# revision 21
# speedup vs baseline: 1.0846x; 1.0521x over previous
"""CrossVarianceAttention Trainium2 kernel.

Sharding: data-parallel over batch B=8, one batch element per NeuronCore
(8 cores). Each core computes the full two-branch cross-attention for its
batch element; outputs are gathered (and transposed) on host.

Device layout notes (per core, one batch element):
  - activations [1024, 512] are transposed on-device to [512, 1024]
    ("T layout": feature on partitions) because every matmul contracts
    over features.
  - attention runs per (branch, head) in [k, q] layout (k on partitions)
    so that att @ V needs no transpose; per-q statistics (mean/var of the
    variance-weighting and the softmax denominator) are computed with
    tensor-engine ones/B-matrix reductions, then broadcast across
    partitions with gpsimd partition_broadcast.
  - final out_proj produces out^T [512, 1024]; host transposes back.
"""

import os
import sys
from contextlib import ExitStack

import numpy as np

for _p in ("/opt/trn_rl_repo", "/root/.axon_site/_ro/trn_rl_repo"):
    if os.path.isdir(_p) and _p not in sys.path:
        sys.path.insert(0, _p)

import concourse.bass as bass
import concourse.bacc as bacc
import concourse.mybir as mybir
from concourse import tile
from concourse.bass_utils import run_bass_kernel_spmd
from concourse.masks import make_identity

F32 = mybir.dt.float32
F16 = mybir.dt.float16
BF16 = mybir.dt.bfloat16
AX = mybir.AxisListType
OP = mybir.AluOpType
AF = mybir.ActivationFunctionType

B, N, D = 8, 1024, 512
H, DK = 8, 64
NT = N // 128          # 8 n/k tiles of 128
DB = D // 128          # 4 feature blocks of 128
CH = N // 512          # 2 free-dim chunks of 512 (fp32 matmul N limit)
SCALE = float(np.sqrt(DK))
LN_EPS = 1e-5

W_NAMES = ["q_vis", "k_vis", "v_vis", "q_ir", "k_ir", "v_ir", "out_vis", "out_ir"]


def _emit(ctx: ExitStack, tc: "tile.TileContext", io: dict):
    nc = tc.nc

    const_pool = ctx.enter_context(tc.tile_pool(name="const", bufs=1))
    ident = const_pool.tile([128, 128], F32)
    make_identity(nc, ident[:])
    ones_f16 = const_pool.tile([128, 1], F16)
    nc.vector.memset(ones_f16[:], 1.0)
    ones_bf = const_pool.tile([128, 1], BF16)
    nc.vector.memset(ones_bf[:], 1.0)
    c_eps = const_pool.tile([128, 1], F32)
    nc.vector.memset(c_eps[:], LN_EPS)
    c_half = const_pool.tile([128, 1], F32)
    nc.vector.memset(c_half[:], 0.5)

    # --- per-feature vectors as [128, DB] columns, one packed DMA ---
    cols_sb = const_pool.tile([128, len(VEC_NAMES), DB], F32, name="cols_sb")
    nc.sync.dma_start(cols_sb[:], io["cols_all"][:])
    cols = {nm: cols_sb[:, i, :] for i, nm in enumerate(VEC_NAMES)}

    # persistent projection outputs
    projT_pool = ctx.enter_context(tc.tile_pool(name="projT", bufs=1))
    QT = {}   # [128, DB, N] f16 : Q^T/SCALE per branch (branch -> tile)
    KT = {}   # [128, DB, N] f16 : K^T per branch
    Vn = {}   # [128, NT, D] bf16: V natural per branch
    for br in ("vis", "ir"):
        QT[br] = projT_pool.tile([128, DB, N], F16, tag=f"QT_{br}", name=f"QT_{br}")
        KT[br] = projT_pool.tile([128, DB, N], F16, tag=f"KT_{br}", name=f"KT_{br}")
        Vn[br] = projT_pool.tile([128, NT, D], BF16, tag=f"V_{br}", name=f"V_{br}")

    ot_pool = ctx.enter_context(tc.tile_pool(name="ot", bufs=1))

    stats_pool = ctx.enter_context(tc.tile_pool(name="stats", bufs=1))
    mu16 = {br: stats_pool.tile([H, N], F16, tag=f"mu16_{br}", name=f"mu16_{br}") for br in ("vis", "ir")}

    # ============ Stage A: LN (gamma/beta folded into W) + DMA-xbar T ======
    with ExitStack() as sctx:
        inT_pool = sctx.enter_context(tc.tile_pool(name="inT", bufs=1))
        xT = {}
        for nm in ("lnT_vis", "lnT_ir", "fusT_rgb", "fusT_ir"):
            xT[nm] = inT_pool.tile([128, DB, N], F16, tag=nm, name=nm)

        a_pool = sctx.enter_context(tc.tile_pool(name="stA", bufs=4))
        st_pool = sctx.enter_context(tc.tile_pool(name="stA_stats", bufs=8))

        def ln_transpose(src_ap, dst):
            # xhat = (x - mean) * rstd  (gamma/beta folded into K/V weights)
            xfull = a_pool.tile([128, NT, D], F32, tag="x_full", bufs=2)
            nc.sync.dma_start(xfull[:], src_ap[:])
            for nt in range(NT):
                x = xfull[:, nt, :]
                ssum = st_pool.tile([128, 1], F32, tag="ssum")
                nc.vector.tensor_reduce(ssum[:], x[:], AX.X, OP.add)
                sq = a_pool.tile([128, D], F32, tag="sq_scratch")
                sqsum = st_pool.tile([128, 1], F32, tag="sqsum")
                nc.scalar.activation(sq[:], x[:], AF.Square, accum_out=sqsum[:])
                mu = st_pool.tile([128, 1], F32, tag="mu")
                nc.vector.tensor_scalar_mul(mu[:], ssum[:], 1.0 / D)
                ex2 = st_pool.tile([128, 1], F32, tag="ex2")
                nc.vector.tensor_scalar_mul(ex2[:], sqsum[:], 1.0 / D)
                mu2 = st_pool.tile([128, 1], F32, tag="mu2")
                nc.vector.tensor_mul(mu2[:], mu[:], mu[:])
                var = st_pool.tile([128, 1], F32, tag="var")
                nc.vector.tensor_sub(var[:], ex2[:], mu2[:])
                std = st_pool.tile([128, 1], F32, tag="std")
                nc.scalar.activation(std[:], var[:], AF.Sqrt, bias=c_eps[:])
                rstd = st_pool.tile([128, 1], F32, tag="rstd")
                nc.vector.reciprocal(rstd[:], std[:])
                xh = a_pool.tile([128, D], F16, tag="xhat")
                nc.vector.tensor_scalar(
                    xh[:], x[:], mu[:], rstd[:], OP.subtract, OP.mult
                )
                nc.sync.dma_start_transpose(
                    dst[:, :, nt * 128:(nt + 1) * 128], xh[:]
                )

        def plain_transpose(src_ap, dst):
            xfull = a_pool.tile([128, NT, D], F32, tag="x_full", bufs=2)
            nc.sync.dma_start(xfull[:], src_ap[:])
            for nt in range(NT):
                x16 = a_pool.tile([128, D], F16, tag="xhat")
                nc.vector.tensor_copy(x16[:], xfull[:, nt, :])
                nc.sync.dma_start_transpose(
                    dst[:, :, nt * 128:(nt + 1) * 128], x16[:]
                )

        ln_transpose(io["rgb_fea"][:], xT["lnT_vis"])
        ln_transpose(io["ir_fea"][:], xT["lnT_ir"])
        plain_transpose(io["rgb_fused"][:], xT["fusT_rgb"])
        plain_transpose(io["ir_fused"][:], xT["fusT_ir"])

        # =================== Stage B: projections (fp16) ===================
        w_pool = sctx.enter_context(tc.tile_pool(name="wts", bufs=2))
        w16_pool = sctx.enter_context(tc.tile_pool(name="wts16", bufs=2))
        pj_pool = sctx.enter_context(
            tc.tile_pool(name="stB_psum", bufs=4, space="PSUM")
        )

        def load_w16(name, gcol=None):
            # fp16 weight; optionally scaled by LN gamma (per-din rows)
            w = w_pool.tile([128, DB, D], F32, tag="W")
            nc.sync.dma_start(w[:], io["W_all"][:, W_NAMES.index(name), :, :])
            w16 = w16_pool.tile([128, DB, D], F16, tag="W16")
            for kb in range(DB):
                if gcol is None:
                    nc.vector.tensor_copy(w16[:, kb, :], w[:, kb, :])
                else:
                    nc.vector.tensor_scalar_mul(
                        w16[:, kb, :], w[:, kb, :], gcol[:, kb:kb + 1]
                    )
            return w, w16

        def fold_beta_bias(wname, w32, bcol_beta):
            # returns [128, DB] col tile of (b_w + beta^T W); column m of the
            # result is computed as W[:, m-tile]^T beta via matmul (keeps the
            # dout index on partitions, so no cross-partition moves needed)
            btot = const_pool.tile([128, DB], F32, tag=f"btot_{wname}",
                                   name=f"btot_{wname}")
            for m in range(DB):
                bw_ps = pj_pool.tile([128, 1], F32, tag="bw_ps", name="bw_ps",
                                     bufs=2)
                for kb in range(DB):
                    nc.tensor.matmul(
                        bw_ps[:],
                        w32[:, kb, m * 128:(m + 1) * 128],
                        bcol_beta[:, kb:kb + 1],
                        start=(kb == 0),
                        stop=(kb == DB - 1),
                    )
                nc.vector.tensor_tensor(
                    btot[:, m:m + 1], bw_ps[:], cols["b_" + wname][:, m:m + 1],
                    OP.add,
                )
            return btot

        def proj_T(xt, w16, bcol, dst, scale=None):
            for m in range(DB):
                for c in range(CH):
                    ps = pj_pool.tile([128, 512], F32, tag="pj")
                    for kb in range(DB):
                        nc.tensor.matmul(
                            ps[:],
                            w16[:, kb, m * 128:(m + 1) * 128],
                            xt[:, kb, c * 512:(c + 1) * 512],
                            start=(kb == 0),
                            stop=(kb == DB - 1),
                        )
                    if scale is None:
                        nc.vector.tensor_scalar_add(
                            dst[:, m, c * 512:(c + 1) * 512], ps[:],
                            bcol[:, m:m + 1]
                        )
                    else:
                        nc.vector.tensor_scalar(
                            dst[:, m, c * 512:(c + 1) * 512],
                            ps[:],
                            bcol[:, m:m + 1],
                            float(scale),
                            OP.add,
                            OP.mult,
                        )

        def proj_N(xt, w16, dst):
            # natural-layout projection (for V); bias folded into out-proj
            for nt in range(NT):
                ps = pj_pool.tile([128, 512], F32, tag="pj")
                for kb in range(DB):
                    nc.tensor.matmul(
                        ps[:],
                        xt[:, kb, nt * 128:(nt + 1) * 128],
                        w16[:, kb, :],
                        start=(kb == 0),
                        stop=(kb == DB - 1),
                    )
                nc.vector.tensor_copy(dst[:, nt, :], ps[:])

        # branch "vis": Q from ir_fused (W_q_ir), K/V from LN(rgb_fea)
        # branch "ir":  Q from rgb_fused (W_q_vis), K/V from LN(ir_fea)
        for br, qw, qx, kvg, kvb, kw, vw, kvx in (
            ("vis", "q_ir", "fusT_ir", "ln1_g", "ln1_b", "k_vis", "v_vis",
             "lnT_vis"),
            ("ir", "q_vis", "fusT_rgb", "ln2_g", "ln2_b", "k_ir", "v_ir",
             "lnT_ir"),
        ):
            _, wq16 = load_w16(qw)
            proj_T(xT[qx], wq16, cols["b_" + qw], QT[br], scale=1.0 / SCALE)
            wk32, wk16 = load_w16(kw, gcol=cols[kvg])
            btot_k = fold_beta_bias(kw, wk32, cols[kvb])
            proj_T(xT[kvx], wk16, btot_k, KT[br])
            wv32, wv16 = load_w16(vw, gcol=cols[kvg])
            # V bias: (b_v + beta^T W_v) folded into out-proj via W_out^T b
            btot_v = fold_beta_bias(vw, wv32, cols[kvb])
            proj_N(xT[kvx], wv16, Vn[br])
            cols["btot_v_" + br] = btot_v

        # ---- Stage B2: column means of scores via B-matrix trick ----
        # mu[h, q] = (sum_k e[k, q]) / N = (ksum_head . Q^T_head)[q] / N
        b2_pool = sctx.enter_context(tc.tile_pool(name="stB2", bufs=1))
        mu_ps_pool = sctx.enter_context(
            tc.tile_pool(name="stB2_psum", bufs=1, space="PSUM")
        )
        for br in ("vis", "ir"):
            ks = b2_pool.tile([128, DB], F32, tag="ksum")
            for kb in range(DB):
                nc.vector.tensor_reduce(
                    ks[:, kb:kb + 1], KT[br][:, kb, :], AX.X, OP.add
                )
            bmat = b2_pool.tile([128, DB, H], F16, tag="bmat")
            nc.vector.memset(bmat[:], 0.0)
            for h in range(H):
                kb_h, base = h // 2, (h % 2) * 64
                nc.vector.tensor_copy(
                    bmat[base:base + 64, kb_h, h:h + 1],
                    ks[base:base + 64, kb_h:kb_h + 1],
                )
            mps = mu_ps_pool.tile([H, N], F32, tag="mu_ps")
            for c in range(CH):
                for kb in range(DB):
                    nc.tensor.matmul(
                        mps[:, c * 512:(c + 1) * 512],
                        bmat[:, kb, :],
                        QT[br][:, kb, c * 512:(c + 1) * 512],
                        start=(kb == 0),
                        stop=(kb == DB - 1),
                    )
            nc.vector.tensor_scalar_mul(mu16[br][:], mps[:], 1.0 / N)

    # =================== Stage C: attention per (branch, head) ============
    OT = {br: ot_pool.tile([128, DB, N], F16, tag=f"OT_{br}", name=f"OT_{br}") for br in ("vis", "ir")}
    cctx = ctx.enter_context(ExitStack())
    c_pool = cctx.enter_context(tc.tile_pool(name="stC", bufs=2))
    c2_pool = cctx.enter_context(tc.tile_pool(name="stC_e", bufs=6))
    bmu_pool = cctx.enter_context(tc.tile_pool(name="stC_bmu", bufs=4))
    row_pool = cctx.enter_context(tc.tile_pool(name="stC_rows", bufs=3))
    eps_pool = cctx.enter_context(tc.tile_pool(name="e_psum", bufs=3, space="PSUM"))
    aux_pool = cctx.enter_context(tc.tile_pool(name="aux_psum", bufs=4, space="PSUM"))
    o_pool = cctx.enter_context(tc.tile_pool(name="o_psum", bufs=1, space="PSUM"))

    def head_ctx(br, h):
        kb_h, base = h // 2, (h % 2) * 64
        return (KT[br][base:base + 64, kb_h, :], QT[br][base:base + 64, kb_h, :],
                kb_h, base)

    HN = 512  # q-half width: chains run per (head, q-half) for deep pipelining

    for br in ("vis", "ir"):
        for j in range(H // 2):
            pair = (2 * j, 2 * j + 1)
            bmu = {}
            for h in pair:
                for g in range(2):
                    mu_row = row_pool.tile([1, HN], F16, tag="mu_row")
                    nc.sync.dma_start(
                        mu_row[:], mu16[br][h:h + 1, g * HN:(g + 1) * HN]
                    )
                    bmu[h, g] = bmu_pool.tile([128, HN], F16, tag="bmu",
                                              name="bmu")
                    nc.gpsimd.partition_broadcast(bmu[h, g][:], mu_row[:])

            for g in range(2):
                qs = slice(g * HN, (g + 1) * HN)
                # --- scores, pair-interleaved for PE row-group overlap ---
                e16 = {h: c2_pool.tile([128, NT, HN], F16, tag="e16",
                                       name="e16") for h in pair}
                for kt in range(NT):
                    eps = {}
                    for h in pair:
                        kt_h, qt_h, _, _ = head_ctx(br, h)
                        eps[h] = eps_pool.tile([128, HN], F32, tag="e_ps",
                                               name="e_ps")
                        nc.tensor.matmul(
                            eps[h][:],
                            kt_h[:, kt * 128:(kt + 1) * 128],
                            qt_h[:, qs],
                            start=True,
                            stop=True,
                        )
                    for h in pair:
                        nc.scalar.copy(e16[h][:, kt, :], eps[h][:])

                # --- in-place chain ---
                tcx = {h: c_pool.tile([128, NT, HN], F16, tag="tc", name="tc",
                                      bufs=4) for h in pair}
                for h in pair:
                    bmu_b = bmu[h, g][:].rearrange(
                        "p (o n) -> p o n", o=1).to_broadcast([128, NT, HN])
                    nc.vector.tensor_sub(tcx[h][:], e16[h][:], bmu_b)
                for h in pair:
                    nc.vector.tensor_mul(tcx[h][:], tcx[h][:], tcx[h][:])
                brx = {}
                for h in pair:
                    vps = aux_pool.tile([1, HN], F32, tag="red_ps",
                                        name="red_ps")
                    for kt in range(NT):
                        nc.tensor.matmul(
                            vps[:],
                            ones_f16[:],
                            tcx[h][:, kt, :],
                            start=(kt == 0),
                            stop=(kt == NT - 1),
                        )
                    rr = row_pool.tile([1, HN], F32, tag="rr")
                    nc.vector.tensor_scalar(
                        rr[:], vps[:], 2.0 / N, 1e-6, OP.mult, OP.add
                    )
                    rf = row_pool.tile([1, HN], F32, tag="rf")
                    nc.vector.reciprocal_approx_fast(rf[:], rr[:])
                    r16row = row_pool.tile([1, HN], F16, tag="r16row")
                    nc.vector.tensor_copy(r16row[:], rf[:])
                    brx[h] = c_pool.tile([128, HN], F16, tag="br16",
                                         name="br16", bufs=4)
                    nc.gpsimd.partition_broadcast(brx[h][:], r16row[:])
                for h in pair:
                    br_b = brx[h][:].rearrange(
                        "p (o n) -> p o n", o=1).to_broadcast([128, NT, HN])
                    nc.vector.tensor_mul(tcx[h][:], tcx[h][:], br_b)
                for h in pair:
                    nc.scalar.activation(tcx[h][:], tcx[h][:], AF.Sigmoid,
                                         bias=c_half[:])
                for h in pair:
                    nc.vector.tensor_mul(tcx[h][:], e16[h][:], tcx[h][:])
                ew = {}
                for h in pair:
                    ew[h] = c2_pool.tile([128, NT, HN], BF16, tag="e16",
                                         name="ew")
                    nc.scalar.activation(ew[h][:], tcx[h][:], AF.Exp)

                # --- softmax denominator ---
                brd = {}
                for h in pair:
                    dps = aux_pool.tile([1, HN], F32, tag="red_ps",
                                        name="red_ps")
                    for kt in range(NT):
                        nc.tensor.matmul(
                            dps[:],
                            ones_bf[:],
                            ew[h][:, kt, :],
                            start=(kt == 0),
                            stop=(kt == NT - 1),
                        )
                    dd = row_pool.tile([1, HN], F32, tag="rr")
                    nc.vector.tensor_copy(dd[:], dps[:])
                    rd = row_pool.tile([1, HN], F32, tag="rf")
                    nc.vector.reciprocal_approx_fast(rd[:], dd[:])
                    brd[h] = c_pool.tile([128, HN], F32, tag="brd",
                                         name="brd", bufs=3)
                    nc.gpsimd.partition_broadcast(brd[h][:], rd[:])

                # --- AV, pair-packed into psum column groups ---
                ops = o_pool.tile([128, HN], F32, tag="o_ps", name="o_ps")
                for kt in range(NT):
                    for h in pair:
                        base_o = (h % 2) * 64
                        nc.tensor.matmul(
                            ops[base_o:base_o + 64, :],
                            Vn[br][:, kt, h * 64:(h + 1) * 64],
                            ew[h][:, kt, :],
                            start=(kt == 0),
                            stop=(kt == NT - 1),
                            tile_position=(0, base_o),
                            skip_group_check=True,
                        )
                for h in pair:
                    _, _, kb_h, base = head_ctx(br, h)
                    base_o = (h % 2) * 64
                    nc.vector.scalar_tensor_tensor(
                        OT[br][base:base + 64, kb_h, qs],
                        ops[base_o:base_o + 64, :],
                        1.0,
                        brd[h][:64, :],
                        OP.mult,
                        OP.mult,
                    )

    cctx.close()

    # =================== Stage D: out-proj (transposed output) ============
    with ExitStack() as sctx:
        w_pool = sctx.enter_context(tc.tile_pool(name="wts_out", bufs=2))
        d_pool = sctx.enter_context(tc.tile_pool(name="stD", bufs=4))
        dp_pool = sctx.enter_context(
            tc.tile_pool(name="stD_psum", bufs=4, space="PSUM")
        )
        for br in ("vis", "ir"):
            wname = "out_" + br
            w32 = w_pool.tile([128, DB, D], F32, tag="Wout32")
            nc.sync.dma_start(w32[:], io["W_all"][:, W_NAMES.index(wname), :, :])
            w = w_pool.tile([128, DB, D], F16, tag="Wout")
            nc.vector.tensor_copy(w[:], w32[:])
            bout = cols["b_" + wname]
            bv = cols["btot_v_" + br]
            # total bias = b_out + W_out^T b_v   (V-projection bias folded in)
            btot = d_pool.tile([128, DB], F32, tag="btot")
            for m in range(DB):
                wb = dp_pool.tile([128, 1], F32, tag="wb_ps")
                for kb in range(DB):
                    nc.tensor.matmul(
                        wb[:],
                        w32[:, kb, m * 128:(m + 1) * 128],
                        bv[:, kb:kb + 1],
                        start=(kb == 0),
                        stop=(kb == DB - 1),
                    )
                nc.vector.tensor_add(btot[:, m:m + 1], wb[:], bout[:, m:m + 1])
            out_dram = io["out_vis_pm"] if br == "vis" else io["out_ir_pm"]
            ostage = d_pool.tile([128, DB, CH, 512], F32, tag="ostage", bufs=2)
            for m in range(DB):
                for c in range(CH):
                    ps = dp_pool.tile([128, 512], F32, tag="op_ps")
                    for kb in range(DB):
                        nc.tensor.matmul(
                            ps[:],
                            w[:, kb, m * 128:(m + 1) * 128],
                            OT[br][:, kb, c * 512:(c + 1) * 512],
                            start=(kb == 0),
                            stop=(kb == DB - 1),
                        )
                    nc.vector.tensor_scalar_add(
                        ostage[:, m, c, :], ps[:], btot[:, m:m + 1]
                    )
            nc.sync.dma_start(out_dram[:], ostage[:])


VEC_NAMES = ["b_" + nm for nm in W_NAMES] + ["ln1_g", "ln1_b", "ln2_g", "ln2_b"]


def build_nc():
    nc = bacc.Bacc()
    io = {}
    # activations arrive partition-major: [128, NT, D]
    for nm in ["rgb_fea", "ir_fea", "rgb_fused", "ir_fused"]:
        io[nm] = nc.declare_dram_parameter(nm, [128, NT, D], F32, isOutput=False)
    # weights partition-major: [128, NW, DB, D]; per-feature vectors as
    # [128, NV, DB] column stacks
    io["W_all"] = nc.declare_dram_parameter(
        "W_all", [128, len(W_NAMES), DB, D], F32, isOutput=False)
    io["cols_all"] = nc.declare_dram_parameter(
        "cols_all", [128, len(VEC_NAMES), DB], F32, isOutput=False)
    io["out_vis_pm"] = nc.declare_dram_parameter(
        "out_vis_pm", [128, DB, CH, 512], F32, isOutput=True)
    io["out_ir_pm"] = nc.declare_dram_parameter(
        "out_ir_pm", [128, DB, CH, 512], F32, isOutput=True)

    with tile.TileContext(nc) as tc:
        with ExitStack() as ctx:
            _emit(ctx, tc, io)
    nc.finalize()
    return nc


_NC_CACHE = None


def _get_nc():
    global _NC_CACHE
    if _NC_CACHE is None:
        _NC_CACHE = build_nc()
    return _NC_CACHE


def _pm(x):
    # [1024, 512] -> [128, NT, D] partition-major
    return np.ascontiguousarray(
        np.asarray(x, np.float32).reshape(NT, 128, D).transpose(1, 0, 2))


def _in_maps(rgb_fea, ir_fea, rgb_fused, ir_fused, params):
    w_all = np.ascontiguousarray(
        np.stack([np.asarray(params["W_" + nm], np.float32)
                  .reshape(DB, 128, D).transpose(1, 0, 2)
                  for nm in W_NAMES], axis=1))  # [128, NW, DB, D]
    cols_all = np.ascontiguousarray(
        np.stack([np.asarray(params[nm], np.float32).reshape(DB, 128).T
                  for nm in VEC_NAMES], axis=1))  # [128, NV, DB]
    maps = []
    for i in range(len(rgb_fea)):
        maps.append({
            "rgb_fea": _pm(rgb_fea[i]),
            "ir_fea": _pm(ir_fea[i]),
            "rgb_fused": _pm(rgb_fused[i]),
            "ir_fused": _pm(ir_fused[i]),
            "W_all": w_all,
            "cols_all": cols_all,
        })
    return maps


def run(rgb_fea, ir_fea, rgb_fused, ir_fused, params, trace=False):
    nc = _get_nc()
    maps = _in_maps(
        np.asarray(rgb_fea), np.asarray(ir_fea),
        np.asarray(rgb_fused), np.asarray(ir_fused), params,
    )
    res = run_bass_kernel_spmd(nc, maps, list(range(B)), trace=trace)

    def _unstage(a):
        # [128, DB, CH, 512] -> [1024, 512]: out[q=(c,i), dout=(m,p)]
        return a.transpose(2, 3, 1, 0).reshape(N, D)

    out_vis = np.stack([_unstage(res.results[i]["out_vis_pm"]) for i in range(B)])
    out_ir = np.stack([_unstage(res.results[i]["out_ir_pm"]) for i in range(B)])
    return (out_vis, out_ir), res


def kernel(rgb_fea, ir_fea, rgb_fused, ir_fused, params):
    (out_vis, out_ir), _ = run(rgb_fea, ir_fea, rgb_fused, ir_fused, params)
    return out_vis, out_ir
